# revision 15
# baseline (speedup 1.0000x reference)
"""Trainium2 Bass kernel for the batched attention module:

    proj   = input @ W.T + b            # [B, TQ, 2H]
    scores = proj @ enc                 # [B, TQ, S]   (enc: [B, 2H, S], S == 2H)
    attn   = softmax(scores, axis=-1)
    out    = attn @ enc                 # [B, TQ, S]

Sharding: data-parallel over batch, one batch per NeuronCore (8 cores).

Dtypes: the scores/out matmuls run as f32r x f32r (fp32 stored, fp22
multiplied) -- measured 227ns per 512-col matmul.  An all-16-bit
variant measured 259ns/MM: sustained dense 16-bit matmul work trips
the chip's power throttle (~200us SW loop) and downclocks the PE from
2.4 to ~2.0GHz for the rest of the kernel, so f32r is the fastest
usable dtype for the big accumulating matmuls (and bf16/fp16 enc would
blow the 2e-2 gate anyway: softmax amplifies absolute score error).
P1 runs fp16 x fp16 in short bursts (numerically safe: proj quant
error ~4e-4; measured 216ns/MM) so the front DMA is 6MB instead of
18MB -- the 16MB fp32 enc load then hides behind P1(g0)'s PE work.
E is written fp16 by the ACT exp, so the 128x128 PE transposes run at
1cyc/row with 8 blocks packed per PSUM bank.  Measured rel err 2.2e-3
(gate 2e-2).

Dataflow per core (batch):
  Front queue: inp[g0], wt0..15, enc0..15, inp[g1]; P1(g0) starts
  ~2us in and covers most of the enc load; scores(0) trails the last
  enc tiles.  W is re-streamed for P1(g1) at iteration 3 (queue is
  idle then); P1(g1) cannot run earlier: its projT eviction reuses
  the single projT slot whose readers scores(0..3) must already be
  emitted (deadlock otherwise).
  Steady state per q-tile i (PE FIFO order):
      out(i-1) c2,c3 | transp(i) | [P1(g1) before scores(4)] |
      scores(i+1) | out(i) c0,c1
  so the softmax (DVE max -> ACT exp -> DVE recip) of tile i hides
  under ~14us of out(i-1) matmuls, and the scores PSUM banks are free
  (exp has read them) before scores(i+1) starts.  P1's per-d-tile
  accumulators alternate between the ps_small and ps_out pools (4-bank
  rotation) so DVE evictions never gate the next matmul group.  The
  final output chunk is computed in two 256-col halves so the last
  store starts earlier.
"""

import sys

import numpy as np

for _p in ("/opt/trn_rl_repo",):
    if _p not in sys.path:
        sys.path.insert(0, _p)

from concourse import bacc, bass, mybir, tile  # noqa: E402
from concourse.bass_utils import run_bass_kernel_spmd  # noqa: E402
from concourse.masks import make_identity  # noqa: E402

F32 = mybir.dt.float32
F32R = mybir.dt.float32r
F16 = mybir.dt.float16
AF = mybir.ActivationFunctionType
AX = mybir.AxisListType
ALU = mybir.AluOpType


def r32(ap):
    return ap.bitcast(F32R)


B = 8
TQ = 1024
H = 1024
D = 2 * H  # 2048, proj feature dim == contraction dim of scores
S = 2 * H  # 2048
P = 128

NHT = H // P  # 8  h-tiles
NDT = D // P  # 16 d-tiles
NST = S // P  # 16 s-tiles
NQT = TQ // P  # 8 q-tiles
QG = 512  # q-group width for the proj phase
NG = TQ // QG  # 2 groups
QTPG = QG // P  # 4 q-tiles per group
NCH = 512  # moving-dim chunk for scores/out matmuls (one PSUM bank of fp32)
NSC = S // NCH  # 4


def build_program() -> bass.Bass:
    nc = bacc.Bacc(
        "TRN2",
        target_bir_lowering=False,
        debug=False,
        # SWDGE (gpsimd) casting DMAs stream enc; they need descriptor
        # ring scratch
        dynamic_dma_scratch_size=4096,
    )
    inpT = nc.declare_dram_parameter("inpT", [NG, P, NHT, QG], F16, isOutput=False)
    wt = nc.declare_dram_parameter("wt", [NDT, P, NHT, P], F16, isOutput=False)
    enc = nc.declare_dram_parameter("enc", [S, S], F16, isOutput=False)
    bvec = nc.declare_dram_parameter("bvec", [P, NDT], F32, isOutput=False)
    out = nc.declare_dram_parameter("out", [TQ, S], F32, isOutput=True)

    with tile.TileContext(nc) as tc:
        with (
            tc.tile_pool(name="const", bufs=1) as constp,
            tc.tile_pool(name="inp", bufs=1) as inpp,
            tc.tile_pool(name="wtp", bufs=4) as wtp,
            tc.tile_pool(name="projp", bufs=1) as projp,
            tc.tile_pool(name="ep", bufs=2) as ep,
            tc.tile_pool(name="etp", bufs=2) as etp,
            tc.tile_pool(name="outp", bufs=2) as outp,
            tc.tile_pool(name="statp", bufs=2) as statp,
            tc.tile_pool(name="ps_sc", bufs=1, space="PSUM") as ps_sc,
            tc.tile_pool(name="ps_small", bufs=2, space="PSUM") as ps_small,
            tc.tile_pool(name="ps_out", bufs=2, space="PSUM") as ps_out,
        ):
            identh = constp.tile([P, P], F16)
            make_identity(nc, identh[:])
            bias_sb = constp.tile([P, NDT], F32)
            nc.scalar.dma_start(out=bias_sb[:], in_=bvec[:])

            # per-group input tiles (fp16, 8KB/partition each); group 1's
            # DMA is enqueued after enc (P1(g1) runs late)
            inp_g0 = inpp.tile([P, NHT, QG], F16, tag="inp0")
            inp_g1 = inpp.tile([P, NHT, QG], F16, tag="inp1")
            inp_g = [inp_g0, inp_g1]

            def emit_p1(g):
                projT = projp.tile([P, NDT, QG], F32R, tag="projT")
                for dt_ in range(NDT):
                    wt_sl = wtp.tile([P, NHT, P], F16, tag="wt")
                    nc.sync.dma_start(out=wt_sl[:], in_=wt[dt_])
                    if dt_ % 2 == 0:
                        pp = ps_small.tile([P, QG], F32, tag="small")
                    else:
                        pp = ps_out.tile([P, QG], F32, tag="po")
                    for ht in range(NHT):
                        nc.tensor.matmul(
                            pp[:],
                            wt_sl[:, ht, :],
                            inp_g[g][:, ht, :],
                            start=(ht == 0),
                            stop=(ht == NHT - 1),
                        )
                    nc.vector.tensor_scalar_add(
                        projT[:, dt_, :], pp[:], bias_sb[:, dt_ : dt_ + 1]
                    )
                return projT

            # ---------------- front: P1(g0) over the enc load
            # enc ships fp16 (8MB instead of 16MB -- the front is
            # DMA-bound and the 50%-duty HW throttle tracks the DMA
            # burst) and is widened to f32 in-flight by SWDGE casting
            # DMAs on the gpsimd queue, decoupled from the inp/wt
            # stream on the sync queue.  The f32 copy keeps the big
            # matmuls off the 16-bit power-throttle path.  inp[g0]
            # lands in two halves so P1's first matmuls start earlier.
            _mark(nc, "front")
            nc.sync.dma_start(
                out=inp_g[0][:, 0 : NHT // 2, :], in_=inpT[0, :, 0 : NHT // 2]
            )
            nc.sync.dma_start(
                out=inp_g[0][:, NHT // 2 :, :], in_=inpT[0, :, NHT // 2 :]
            )
            projs = {0: emit_p1(0)}

            enc_sb = []
            for st_ in range(NST):
                e = constp.tile([P, S], F32R, tag=f"enc{st_}")
                nc.gpsimd.dma_start(
                    out=e[:], in_=enc[st_ * P : (st_ + 1) * P, :]
                )
                enc_sb.append(e)
            nc.sync.dma_start(out=inp_g[1][:], in_=inpT[1])

            def emit_scores(qt):
                sc = ps_sc.tile([P, S], F32, tag="sc")
                projT = projs[qt // QTPG]
                qs = slice((qt % QTPG) * P, (qt % QTPG + 1) * P)
                for dt_ in range(NDT):
                    for c in range(NSC):
                        cs = slice(c * NCH, (c + 1) * NCH)
                        nc.tensor.matmul(
                            sc[:, cs],
                            projT[:, dt_, qs],
                            r32(enc_sb[dt_][:, cs]),
                            start=(dt_ == 0),
                            stop=(dt_ == NDT - 1),
                        )
                return sc

            def emit_softmax_stats(sc):
                st = statp.tile([P, 8], F32, tag="stat")
                nc.vector.tensor_reduce(
                    st[:, 0:1], sc[:], axis=AX.X, op=ALU.max, negate=True
                )
                E = ep.tile([P, S], F16, tag="E")
                for hf in range(2):
                    hsl = slice(hf * (S // 2), (hf + 1) * (S // 2))
                    nc.scalar.activation(
                        E[:, hsl],
                        sc[:, hsl],
                        AF.Exp,
                        bias=st[:, 0:1],
                        scale=1.0,
                        accum_out=st[:, 4 + hf : 5 + hf],
                    )
                return E, st

            def emit_transp(E):
                ET = etp.tile([P, NST, P], F32R, tag="ET")
                for sb in range(NST // 8):
                    tp = ps_small.tile([P, 8 * P], F16, tag="small")
                    for j in range(8):
                        nc.tensor.transpose(
                            tp[:, j * P : (j + 1) * P],
                            E[:, (8 * sb + j) * P : (8 * sb + j + 1) * P],
                            identh[:],
                        )
                    nc.vector.tensor_copy(ET[:, 8 * sb : 8 * sb + 8, :], tp[:])
                return ET

            def emit_out_chunk(ET, st, qt, c):
                cs = slice(c * NCH, (c + 1) * NCH)
                po = ps_out.tile([P, NCH], F32, tag="po")
                for s_ in range(NST):
                    nc.tensor.matmul(
                        po[:],
                        ET[:, s_, :],
                        r32(enc_sb[s_][:, cs]),
                        start=(s_ == 0),
                        stop=(s_ == NST - 1),
                    )
                ob = outp.tile([P, NCH], F32, tag="ob")
                nc.vector.tensor_scalar_mul(ob[:], po[:], st[:, 2:3])
                nc.sync.dma_start(out=out[qt * P : (qt + 1) * P, cs], in_=ob[:])

            # ---------------- steady state
            _mark(nc, "scores(0)")
            cur_sc = emit_scores(0)
            prev = None  # (ET, st, qt) of tile i-1
            for i in range(NQT):
                _mark(nc, f"softmax({i})")
                E, st = emit_softmax_stats(cur_sc)
                if prev is not None:
                    _mark(nc, f"out({i - 1})c23")
                    emit_out_chunk(*prev, 2)
                    emit_out_chunk(*prev, 3)
                _mark(nc, f"transp({i})")
                ET = emit_transp(E)
                nc.vector.tensor_reduce(
                    st[:, 1:2], st[:, 4:6], axis=AX.X, op=ALU.add
                )
                nc.vector.reciprocal(st[:, 2:3], st[:, 1:2])
                if i + 1 < NQT:
                    if (i + 1) % QTPG == 0:
                        _mark(nc, f"P1({(i + 1) // QTPG})")
                        projs[(i + 1) // QTPG] = emit_p1((i + 1) // QTPG)
                    _mark(nc, f"scores({i + 1})")
                    cur_sc = emit_scores(i + 1)
                _mark(nc, f"out({i})c01")
                emit_out_chunk(ET, st, i, 0)
                emit_out_chunk(ET, st, i, 1)
                prev = (ET, st, i)
            _mark(nc, "tail")
            emit_out_chunk(*prev, 2)
            ETp, stp, qtp = prev
            for half in range(2):
                hs = slice(
                    3 * NCH + half * (NCH // 2),
                    3 * NCH + (half + 1) * (NCH // 2),
                )
                po = ps_out.tile([P, NCH], F32, tag="po")
                for s_ in range(NST):
                    nc.tensor.matmul(
                        po[:, 0 : NCH // 2],
                        ETp[:, s_, :],
                        r32(enc_sb[s_][:, hs]),
                        start=(s_ == 0),
                        stop=(s_ == NST - 1),
                    )
                ob = outp.tile([P, NCH], F32, tag="ob")
                nc.vector.tensor_scalar_mul(
                    ob[:, 0 : NCH // 2], po[:, 0 : NCH // 2], stp[:, 2:3]
                )
                nc.sync.dma_start(
                    out=out[qtp * P : (qtp + 1) * P, hs], in_=ob[:, 0 : NCH // 2]
                )
            _mark(nc, "end")

    nc.compile()
    return nc


PHASES = []  # (instruction id, label) marks populated during build


def _mark(nc, label):
    nm = nc.get_next_instruction_name()  # burns one name; fine
    PHASES.append((int(nm.split("-")[1]), label))


_NC_CACHE = {}


def _get_program(loop_n: int = 1) -> bass.Bass:
    if loop_n not in _NC_CACHE:
        PHASES.clear()
        _NC_CACHE[loop_n] = build_program()
    return _NC_CACHE[loop_n]


def _prep_in_maps(input, encoder_output, W, b):
    input = np.ascontiguousarray(input, dtype=np.float32)
    encoder_output = np.ascontiguousarray(encoder_output, dtype=np.float32)
    W = np.ascontiguousarray(W, dtype=np.float32)
    b = np.ascontiguousarray(b, dtype=np.float32)

    # input per batch [TQ, H] -> inpT[g, hp, ht, q'] = input[g*QG+q', ht*P+hp]
    inpT = np.ascontiguousarray(
        input.reshape(B, NG, QG, NHT, P).transpose(0, 1, 4, 3, 2)
    ).astype(np.float16)
    # W[d, h] -> wt[dt, hp, ht, dj] = W[dt*P+dj, ht*P+hp], fp16
    wt = np.ascontiguousarray(
        W.reshape(NDT, P, NHT, P).transpose(0, 3, 2, 1)
    ).astype(np.float16)
    bvec = np.ascontiguousarray(b.reshape(NDT, P).T)  # [P, NDT] fp32

    enc16 = encoder_output.astype(np.float16)
    return [
        {"inpT": inpT[i], "wt": wt, "enc": enc16[i], "bvec": bvec}
        for i in range(B)
    ]


def _out_from_results(res):
    return np.stack([np.asarray(res.results[i]["out"]) for i in range(B)])


def run(input, encoder_output, W, b, trace=False, loop_n=1):
    """Returns (out [B, TQ, S] float32, BassKernelResults)."""
    nc = _get_program(loop_n)
    in_maps = _prep_in_maps(input, encoder_output, W, b)
    res = run_bass_kernel_spmd(nc, in_maps, list(range(B)), trace=trace)
    return _out_from_results(res), res


def kernel(input, encoder_output, W, b):
    out, _ = run(input, encoder_output, W, b, trace=False)
    return out



# revision 18
# speedup vs baseline: 1.0743x; 1.0743x over previous
"""Trainium2 Bass kernel for the batched attention module:

    proj   = input @ W.T + b            # [B, TQ, 2H]
    scores = proj @ enc                 # [B, TQ, S]   (enc: [B, 2H, S], S == 2H)
    attn   = softmax(scores, axis=-1)
    out    = attn @ enc                 # [B, TQ, S]

Sharding: data-parallel over batch, one batch per NeuronCore (8 cores).

Dtypes: the scores/out matmuls run as f32r x f32r (fp32 stored, fp22
multiplied) -- measured 227ns per 512-col matmul.  An all-16-bit
variant measured 259ns/MM: sustained dense 16-bit matmul work trips
the chip's power throttle (~200us SW loop) and downclocks the PE from
2.4 to ~2.0GHz for the rest of the kernel, so f32r is the fastest
usable dtype for the big accumulating matmuls (and bf16/fp16 enc would
blow the 2e-2 gate anyway: softmax amplifies absolute score error).
P1 runs fp16 x fp16 in short bursts (numerically safe: proj quant
error ~4e-4; measured 216ns/MM) so the front DMA is 6MB instead of
18MB -- the 16MB fp32 enc load then hides behind P1(g0)'s PE work.
E is written fp16 by the ACT exp, so the 128x128 PE transposes run at
1cyc/row with 8 blocks packed per PSUM bank.  Measured rel err 2.2e-3
(gate 2e-2).

Dataflow per core (batch):
  Front queue: inp[g0], wt0..15, enc0..15, inp[g1]; P1(g0) starts
  ~2us in and covers most of the enc load; scores(0) trails the last
  enc tiles.  W is re-streamed for P1(g1) at iteration 3 (queue is
  idle then); P1(g1) cannot run earlier: its projT eviction reuses
  the single projT slot whose readers scores(0..3) must already be
  emitted (deadlock otherwise).
  Steady state per q-tile i (PE FIFO order):
      out(i-1) c2,c3 | transp(i) | [P1(g1) before scores(4)] |
      scores(i+1) | out(i) c0,c1
  so the softmax (DVE max -> ACT exp -> DVE recip) of tile i hides
  under ~14us of out(i-1) matmuls, and the scores PSUM banks are free
  (exp has read them) before scores(i+1) starts.  P1's per-d-tile
  accumulators alternate between the ps_small and ps_out pools (4-bank
  rotation) so DVE evictions never gate the next matmul group.  The
  final output chunk is computed in two 256-col halves so the last
  store starts earlier.
"""

import sys

import numpy as np

for _p in ("/opt/trn_rl_repo",):
    if _p not in sys.path:
        sys.path.insert(0, _p)

from concourse import bacc, bass, mybir, tile  # noqa: E402
from concourse.bass_utils import run_bass_kernel_spmd  # noqa: E402
from concourse.masks import make_identity  # noqa: E402

F32 = mybir.dt.float32
F32R = mybir.dt.float32r
F16 = mybir.dt.float16
AF = mybir.ActivationFunctionType
AX = mybir.AxisListType
ALU = mybir.AluOpType


def r32(ap):
    return ap.bitcast(F32R)


B = 8
TQ = 1024
H = 1024
D = 2 * H  # 2048, proj feature dim == contraction dim of scores
S = 2 * H  # 2048
P = 128

NHT = H // P  # 8  h-tiles
NDT = D // P  # 16 d-tiles
NST = S // P  # 16 s-tiles
NQT = TQ // P  # 8 q-tiles
QG = 512  # q-group width for the proj phase
NG = TQ // QG  # 2 groups
QTPG = QG // P  # 4 q-tiles per group
NCH = 512  # moving-dim chunk for scores/out matmuls (one PSUM bank of fp32)
NSC = S // NCH  # 4


def build_program() -> bass.Bass:
    nc = bacc.Bacc(
        "TRN2",
        target_bir_lowering=False,
        debug=False,
        # only HWDGE queues (sync/scalar) are used; reclaim SWDGE scratch
        dynamic_dma_scratch_size=2048,
    )
    inpT = nc.declare_dram_parameter("inpT", [NG, P, NHT, QG], F16, isOutput=False)
    wt = nc.declare_dram_parameter("wt", [NDT, P, NHT, P], F16, isOutput=False)
    enc = nc.declare_dram_parameter("enc", [S, S], F16, isOutput=False)
    bvec = nc.declare_dram_parameter("bvec", [P, NDT], F32, isOutput=False)
    out = nc.declare_dram_parameter("out", [TQ, S], F32, isOutput=True)

    with tile.TileContext(nc) as tc:
        with (
            tc.tile_pool(name="const", bufs=1) as constp,
            tc.tile_pool(name="inp", bufs=1) as inpp,
            tc.tile_pool(name="wtp", bufs=3) as wtp,
            tc.tile_pool(name="encst", bufs=4) as encst,
            tc.tile_pool(name="projp", bufs=1) as projp,
            tc.tile_pool(name="ep", bufs=2) as ep,
            tc.tile_pool(name="etp", bufs=2) as etp,
            tc.tile_pool(name="outp", bufs=2) as outp,
            tc.tile_pool(name="statp", bufs=2) as statp,
            tc.tile_pool(name="ps_sc", bufs=1, space="PSUM") as ps_sc,
            tc.tile_pool(name="ps_small", bufs=2, space="PSUM") as ps_small,
            tc.tile_pool(name="ps_out", bufs=2, space="PSUM") as ps_out,
        ):
            identh = constp.tile([P, P], F16)
            make_identity(nc, identh[:])
            bias_sb = constp.tile([P, NDT], F32)
            nc.scalar.dma_start(out=bias_sb[:], in_=bvec[:])

            # per-group input tiles (fp16, 8KB/partition each); group 1's
            # DMA is enqueued after enc (P1(g1) runs late)
            inp_g0 = inpp.tile([P, NHT, QG], F16, tag="inp0")
            inp_g1 = inpp.tile([P, NHT, QG], F16, tag="inp1")
            inp_g = [inp_g0, inp_g1]

            def emit_p1(g):
                projT = projp.tile([P, NDT, QG], F32R, tag="projT")
                for dt_ in range(NDT):
                    wt_sl = wtp.tile([P, NHT, P], F16, tag="wt")
                    nc.sync.dma_start(out=wt_sl[:], in_=wt[dt_])
                    if dt_ % 2 == 0:
                        pp = ps_small.tile([P, QG], F32, tag="small")
                    else:
                        pp = ps_out.tile([P, QG], F32, tag="po")
                    for ht in range(NHT):
                        nc.tensor.matmul(
                            pp[:],
                            wt_sl[:, ht, :],
                            inp_g[g][:, ht, :],
                            start=(ht == 0),
                            stop=(ht == NHT - 1),
                        )
                    nc.vector.tensor_scalar_add(
                        projT[:, dt_, :], pp[:], bias_sb[:, dt_ : dt_ + 1]
                    )
                return projT

            # ---------------- front: P1(g0) over the enc load
            # enc ships fp16 (8MB instead of 16MB -- the front is
            # DMA-bound and the 50%-duty HW throttle tracks the DMA
            # burst) and is widened to f32 in-flight by SWDGE casting
            # DMAs on the gpsimd queue, decoupled from the inp/wt
            # stream on the sync queue.  The f32 copy keeps the big
            # matmuls off the 16-bit power-throttle path.  inp[g0]
            # lands in two halves so P1's first matmuls start earlier.
            _mark(nc, "front")
            nc.sync.dma_start(
                out=inp_g[0][:, 0 : NHT // 2, :], in_=inpT[0, :, 0 : NHT // 2]
            )
            nc.sync.dma_start(
                out=inp_g[0][:, NHT // 2 :, :], in_=inpT[0, :, NHT // 2 :]
            )
            projs = {0: emit_p1(0)}

            enc_sb = []
            for st_ in range(NST):
                e = constp.tile([P, S], F32R, tag=f"enc{st_}")
                for hf in range(2):
                    hsl = slice(hf * (S // 2), (hf + 1) * (S // 2))
                    stg = encst.tile([P, S // 2], F16, tag="e16")
                    nc.scalar.dma_start(
                        out=stg[:], in_=enc[st_ * P : (st_ + 1) * P, hsl]
                    )
                    nc.vector.tensor_copy(e[:, hsl], stg[:])
                enc_sb.append(e)
            nc.sync.dma_start(out=inp_g[1][:], in_=inpT[1])

            def emit_scores(qt):
                sc = ps_sc.tile([P, S], F32, tag="sc")
                projT = projs[qt // QTPG]
                qs = slice((qt % QTPG) * P, (qt % QTPG + 1) * P)
                for dt_ in range(NDT):
                    for c in range(NSC):
                        cs = slice(c * NCH, (c + 1) * NCH)
                        nc.tensor.matmul(
                            sc[:, cs],
                            projT[:, dt_, qs],
                            r32(enc_sb[dt_][:, cs]),
                            start=(dt_ == 0),
                            stop=(dt_ == NDT - 1),
                        )
                return sc

            def emit_softmax_stats(sc):
                st = statp.tile([P, 8], F32, tag="stat")
                nc.vector.tensor_reduce(
                    st[:, 0:1], sc[:], axis=AX.X, op=ALU.max, negate=True
                )
                E = ep.tile([P, S], F16, tag="E")
                for hf in range(2):
                    hsl = slice(hf * (S // 2), (hf + 1) * (S // 2))
                    nc.scalar.activation(
                        E[:, hsl],
                        sc[:, hsl],
                        AF.Exp,
                        bias=st[:, 0:1],
                        scale=1.0,
                        accum_out=st[:, 4 + hf : 5 + hf],
                    )
                return E, st

            def emit_transp(E):
                ET = etp.tile([P, NST, P], F32R, tag="ET")
                for sb in range(NST // 8):
                    tp = ps_small.tile([P, 8 * P], F16, tag="small")
                    for j in range(8):
                        nc.tensor.transpose(
                            tp[:, j * P : (j + 1) * P],
                            E[:, (8 * sb + j) * P : (8 * sb + j + 1) * P],
                            identh[:],
                        )
                    nc.vector.tensor_copy(ET[:, 8 * sb : 8 * sb + 8, :], tp[:])
                return ET

            def emit_out_chunk(ET, st, qt, c):
                cs = slice(c * NCH, (c + 1) * NCH)
                po = ps_out.tile([P, NCH], F32, tag="po")
                for s_ in range(NST):
                    nc.tensor.matmul(
                        po[:],
                        ET[:, s_, :],
                        r32(enc_sb[s_][:, cs]),
                        start=(s_ == 0),
                        stop=(s_ == NST - 1),
                    )
                ob = outp.tile([P, NCH], F32, tag="ob")
                nc.vector.tensor_scalar_mul(ob[:], po[:], st[:, 2:3])
                nc.sync.dma_start(out=out[qt * P : (qt + 1) * P, cs], in_=ob[:])

            # ---------------- steady state
            _mark(nc, "scores(0)")
            cur_sc = emit_scores(0)
            prev = None  # (ET, st, qt) of tile i-1
            for i in range(NQT):
                _mark(nc, f"softmax({i})")
                E, st = emit_softmax_stats(cur_sc)
                if prev is not None:
                    _mark(nc, f"out({i - 1})c23")
                    emit_out_chunk(*prev, 2)
                    emit_out_chunk(*prev, 3)
                _mark(nc, f"transp({i})")
                ET = emit_transp(E)
                nc.vector.tensor_reduce(
                    st[:, 1:2], st[:, 4:6], axis=AX.X, op=ALU.add
                )
                nc.vector.reciprocal(st[:, 2:3], st[:, 1:2])
                if i + 1 < NQT:
                    if (i + 1) % QTPG == 0:
                        _mark(nc, f"P1({(i + 1) // QTPG})")
                        projs[(i + 1) // QTPG] = emit_p1((i + 1) // QTPG)
                    _mark(nc, f"scores({i + 1})")
                    cur_sc = emit_scores(i + 1)
                _mark(nc, f"out({i})c01")
                emit_out_chunk(ET, st, i, 0)
                emit_out_chunk(ET, st, i, 1)
                prev = (ET, st, i)
            _mark(nc, "tail")
            emit_out_chunk(*prev, 2)
            ETp, stp, qtp = prev
            for half in range(2):
                hs = slice(
                    3 * NCH + half * (NCH // 2),
                    3 * NCH + (half + 1) * (NCH // 2),
                )
                po = ps_out.tile([P, NCH], F32, tag="po")
                for s_ in range(NST):
                    nc.tensor.matmul(
                        po[:, 0 : NCH // 2],
                        ETp[:, s_, :],
                        r32(enc_sb[s_][:, hs]),
                        start=(s_ == 0),
                        stop=(s_ == NST - 1),
                    )
                ob = outp.tile([P, NCH], F32, tag="ob")
                nc.vector.tensor_scalar_mul(
                    ob[:, 0 : NCH // 2], po[:, 0 : NCH // 2], stp[:, 2:3]
                )
                nc.sync.dma_start(
                    out=out[qtp * P : (qtp + 1) * P, hs], in_=ob[:, 0 : NCH // 2]
                )
            _mark(nc, "end")

    nc.compile()
    return nc


PHASES = []  # (instruction id, label) marks populated during build


def _mark(nc, label):
    nm = nc.get_next_instruction_name()  # burns one name; fine
    PHASES.append((int(nm.split("-")[1]), label))


_NC_CACHE = {}


def _get_program(loop_n: int = 1) -> bass.Bass:
    if loop_n not in _NC_CACHE:
        PHASES.clear()
        _NC_CACHE[loop_n] = build_program()
    return _NC_CACHE[loop_n]


def _prep_in_maps(input, encoder_output, W, b):
    input = np.ascontiguousarray(input, dtype=np.float32)
    encoder_output = np.ascontiguousarray(encoder_output, dtype=np.float32)
    W = np.ascontiguousarray(W, dtype=np.float32)
    b = np.ascontiguousarray(b, dtype=np.float32)

    # input per batch [TQ, H] -> inpT[g, hp, ht, q'] = input[g*QG+q', ht*P+hp]
    inpT = np.ascontiguousarray(
        input.reshape(B, NG, QG, NHT, P).transpose(0, 1, 4, 3, 2)
    ).astype(np.float16)
    # W[d, h] -> wt[dt, hp, ht, dj] = W[dt*P+dj, ht*P+hp], fp16
    wt = np.ascontiguousarray(
        W.reshape(NDT, P, NHT, P).transpose(0, 3, 2, 1)
    ).astype(np.float16)
    bvec = np.ascontiguousarray(b.reshape(NDT, P).T)  # [P, NDT] fp32

    enc16 = encoder_output.astype(np.float16)
    return [
        {"inpT": inpT[i], "wt": wt, "enc": enc16[i], "bvec": bvec}
        for i in range(B)
    ]


def _out_from_results(res):
    return np.stack([np.asarray(res.results[i]["out"]) for i in range(B)])


def run(input, encoder_output, W, b, trace=False, loop_n=1):
    """Returns (out [B, TQ, S] float32, BassKernelResults)."""
    nc = _get_program(loop_n)
    in_maps = _prep_in_maps(input, encoder_output, W, b)
    res = run_bass_kernel_spmd(nc, in_maps, list(range(B)), trace=trace)
    return _out_from_results(res), res


def kernel(input, encoder_output, W, b):
    out, _ = run(input, encoder_output, W, b, trace=False)
    return out



# revision 21
# speedup vs baseline: 1.0858x; 1.0107x over previous
"""Trainium2 Bass kernel for the batched attention module:

    proj   = input @ W.T + b            # [B, TQ, 2H]
    scores = proj @ enc                 # [B, TQ, S]   (enc: [B, 2H, S], S == 2H)
    attn   = softmax(scores, axis=-1)
    out    = attn @ enc                 # [B, TQ, S]

Sharding: data-parallel over batch, one batch per NeuronCore (8 cores).

Dtypes: the scores/out matmuls run as f32r x f32r (fp32 stored, fp22
multiplied) -- measured 227ns per 512-col matmul.  An all-16-bit
variant measured 259ns/MM: sustained dense 16-bit matmul work trips
the chip's power throttle (~200us SW loop) and downclocks the PE from
2.4 to ~2.0GHz for the rest of the kernel, so f32r is the fastest
usable dtype for the big accumulating matmuls (and bf16/fp16 enc would
blow the 2e-2 gate anyway: softmax amplifies absolute score error).
P1 runs fp16 x fp16 in short bursts (numerically safe: proj quant
error ~4e-4; measured 216ns/MM) so the front DMA is 6MB instead of
18MB -- the 16MB fp32 enc load then hides behind P1(g0)'s PE work.
E is written fp16 by the ACT exp, so the 128x128 PE transposes run at
1cyc/row with 8 blocks packed per PSUM bank.  Measured rel err 2.2e-3
(gate 2e-2).

Dataflow per core (batch):
  Front queue: inp[g0], wt0..15, enc0..15, inp[g1]; P1(g0) starts
  ~2us in and covers most of the enc load; scores(0) trails the last
  enc tiles.  W is re-streamed for P1(g1) at iteration 3 (queue is
  idle then); P1(g1) cannot run earlier: its projT eviction reuses
  the single projT slot whose readers scores(0..3) must already be
  emitted (deadlock otherwise).
  Steady state per q-tile i (PE FIFO order):
      out(i-1) c2,c3 | transp(i) | [P1(g1) before scores(4)] |
      scores(i+1) | out(i) c0,c1
  so the softmax (DVE max -> ACT exp -> DVE recip) of tile i hides
  under ~14us of out(i-1) matmuls, and the scores PSUM banks are free
  (exp has read them) before scores(i+1) starts.  P1's per-d-tile
  accumulators alternate between the ps_small and ps_out pools (4-bank
  rotation) so DVE evictions never gate the next matmul group.  The
  final output chunk is computed in two 256-col halves so the last
  store starts earlier.
"""

import sys

import numpy as np

for _p in ("/opt/trn_rl_repo",):
    if _p not in sys.path:
        sys.path.insert(0, _p)

from concourse import bacc, bass, mybir, tile  # noqa: E402
from concourse.bass_utils import run_bass_kernel_spmd  # noqa: E402
from concourse.masks import make_identity  # noqa: E402

F32 = mybir.dt.float32
F32R = mybir.dt.float32r
F16 = mybir.dt.float16
AF = mybir.ActivationFunctionType
AX = mybir.AxisListType
ALU = mybir.AluOpType


def r32(ap):
    return ap.bitcast(F32R)


B = 8
TQ = 1024
H = 1024
D = 2 * H  # 2048, proj feature dim == contraction dim of scores
S = 2 * H  # 2048
P = 128

NHT = H // P  # 8  h-tiles
NDT = D // P  # 16 d-tiles
NST = S // P  # 16 s-tiles
NQT = TQ // P  # 8 q-tiles
QG = 512  # q-group width for the proj phase
NG = TQ // QG  # 2 groups
QTPG = QG // P  # 4 q-tiles per group
NCH = 512  # moving-dim chunk for scores/out matmuls (one PSUM bank of fp32)
NSC = S // NCH  # 4


def build_program() -> bass.Bass:
    nc = bacc.Bacc(
        "TRN2",
        target_bir_lowering=False,
        debug=False,
        # only HWDGE queues (sync/scalar) are used; reclaim SWDGE scratch
        dynamic_dma_scratch_size=2048,
    )
    inpT = nc.declare_dram_parameter("inpT", [NG, P, NHT, QG], F16, isOutput=False)
    wt = nc.declare_dram_parameter("wt", [NDT, P, NHT, P], F16, isOutput=False)
    enc = nc.declare_dram_parameter("enc", [S, S], F16, isOutput=False)
    bvec = nc.declare_dram_parameter("bvec", [P, NDT], F32, isOutput=False)
    out = nc.declare_dram_parameter("out", [TQ, S], F32, isOutput=True)

    with tile.TileContext(nc) as tc:
        with (
            tc.tile_pool(name="const", bufs=1) as constp,
            tc.tile_pool(name="inp", bufs=1) as inpp,
            tc.tile_pool(name="wtp", bufs=4) as wtp,
            tc.tile_pool(name="encst", bufs=4) as encst,
            tc.tile_pool(name="projp", bufs=1) as projp,
            tc.tile_pool(name="ep", bufs=2) as ep,
            tc.tile_pool(name="etp", bufs=2) as etp,
            tc.tile_pool(name="outp", bufs=2) as outp,
            tc.tile_pool(name="statp", bufs=2) as statp,
            tc.tile_pool(name="ps_sc", bufs=1, space="PSUM") as ps_sc,
            tc.tile_pool(name="ps_small", bufs=2, space="PSUM") as ps_small,
            tc.tile_pool(name="ps_out", bufs=2, space="PSUM") as ps_out,
        ):
            identh = constp.tile([P, P], F16)
            make_identity(nc, identh[:])
            bias_sb = constp.tile([P, NDT], F32)
            nc.scalar.dma_start(out=bias_sb[:], in_=bvec[:])

            # per-group input tiles (fp16, 8KB/partition each); group 1's
            # DMA is enqueued after enc (P1(g1) runs late)
            inp_g0 = inpp.tile([P, NHT, QG], F16, tag="inp0")
            inp_g1 = inpp.tile([P, NHT, QG], F16, tag="inp1")
            inp_g = [inp_g0, inp_g1]

            def emit_p1(g):
                projT = projp.tile([P, NDT, QG], F32R, tag="projT")
                for dt_ in range(NDT):
                    wt_sl = wtp.tile([P, NHT, P], F16, tag="wt")
                    nc.sync.dma_start(out=wt_sl[:], in_=wt[dt_])
                    if dt_ % 2 == 0:
                        pp = ps_small.tile([P, QG], F32, tag="small")
                    else:
                        pp = ps_out.tile([P, QG], F32, tag="po")
                    for ht in range(NHT):
                        nc.tensor.matmul(
                            pp[:],
                            wt_sl[:, ht, :],
                            inp_g[g][:, ht, :],
                            start=(ht == 0),
                            stop=(ht == NHT - 1),
                        )
                    nc.vector.tensor_scalar_add(
                        projT[:, dt_, :], pp[:], bias_sb[:, dt_ : dt_ + 1]
                    )
                return projT

            # ---------------- front: P1(g0) + scores(0) over the enc load
            # enc ships fp16 (8MB instead of 16MB: the front is pure
            # HBM-bytes-bound, and the 50%-duty HW throttle tracks the
            # DMA burst) on the otherwise-idle scalar HWDGE queue,
            # decoupled from inp/wt on sync, and is widened to f32r in
            # SBUF by DVE casts (keeps the big matmuls off the 16-bit
            # power-throttle path).  P1(g0) d-tiles, enc tile loads,
            # and scores(0)'s per-d-tile chunks are interleaved (lag 2)
            # so the PE always has issueable work while the two DMA
            # streams share HBM bandwidth.
            _mark(nc, "front")
            nc.sync.dma_start(
                out=inp_g[0][:, 0 : NHT // 2, :], in_=inpT[0, :, 0 : NHT // 2]
            )
            wt0 = wtp.tile([P, NHT, P], F16, tag="wt")
            nc.sync.dma_start(out=wt0[:], in_=wt[0])
            nc.sync.dma_start(
                out=inp_g[0][:, NHT // 2 :, :], in_=inpT[0, :, NHT // 2 :]
            )

            projT0 = projp.tile([P, NDT, QG], F32R, tag="projT")
            sc0 = ps_sc.tile([P, S], F32, tag="sc")
            enc_sb = [None] * NST
            LAG = 2
            for k in range(NDT + LAG):
                if k < NDT:
                    # P1(g0) d-tile k
                    if k == 0:
                        wt_sl = wt0
                    else:
                        wt_sl = wtp.tile([P, NHT, P], F16, tag="wt")
                        nc.sync.dma_start(out=wt_sl[:], in_=wt[k])
                    if k % 2 == 0:
                        pp = ps_small.tile([P, QG], F32, tag="small")
                    else:
                        pp = ps_out.tile([P, QG], F32, tag="po")
                    for ht in range(NHT):
                        nc.tensor.matmul(
                            pp[:],
                            wt_sl[:, ht, :],
                            inp_g[0][:, ht, :],
                            start=(ht == 0),
                            stop=(ht == NHT - 1),
                        )
                    nc.vector.tensor_scalar_add(
                        projT0[:, k, :], pp[:], bias_sb[:, k : k + 1]
                    )
                    # enc tile k: fp16 DMA halves + DVE widening casts
                    e = constp.tile([P, S], F32R, tag=f"enc{k}")
                    for hf in range(2):
                        hsl = slice(hf * (S // 2), (hf + 1) * (S // 2))
                        stg = encst.tile([P, S // 2], F16, tag="e16")
                        nc.scalar.dma_start(
                            out=stg[:], in_=enc[k * P : (k + 1) * P, hsl]
                        )
                        nc.vector.tensor_copy(e[:, hsl], stg[:])
                    enc_sb[k] = e
                if k >= LAG:
                    dt_ = k - LAG
                    for c in range(NSC):
                        cs = slice(c * NCH, (c + 1) * NCH)
                        nc.tensor.matmul(
                            sc0[:, cs],
                            projT0[:, dt_, 0:P],
                            r32(enc_sb[dt_][:, cs]),
                            start=(dt_ == 0),
                            stop=(dt_ == NDT - 1),
                        )
            projs = {0: projT0}
            nc.sync.dma_start(out=inp_g[1][:], in_=inpT[1])

            def emit_scores(qt):
                sc = ps_sc.tile([P, S], F32, tag="sc")
                projT = projs[qt // QTPG]
                qs = slice((qt % QTPG) * P, (qt % QTPG + 1) * P)
                for dt_ in range(NDT):
                    for c in range(NSC):
                        cs = slice(c * NCH, (c + 1) * NCH)
                        nc.tensor.matmul(
                            sc[:, cs],
                            projT[:, dt_, qs],
                            r32(enc_sb[dt_][:, cs]),
                            start=(dt_ == 0),
                            stop=(dt_ == NDT - 1),
                        )
                return sc

            def emit_softmax_stats(sc):
                st = statp.tile([P, 8], F32, tag="stat")
                nc.vector.tensor_reduce(
                    st[:, 0:1], sc[:], axis=AX.X, op=ALU.max, negate=True
                )
                E = ep.tile([P, S], F16, tag="E")
                for hf in range(2):
                    hsl = slice(hf * (S // 2), (hf + 1) * (S // 2))
                    nc.scalar.activation(
                        E[:, hsl],
                        sc[:, hsl],
                        AF.Exp,
                        bias=st[:, 0:1],
                        scale=1.0,
                        accum_out=st[:, 4 + hf : 5 + hf],
                    )
                return E, st

            def emit_transp(E):
                ET = etp.tile([P, NST, P], F32R, tag="ET")
                for sb in range(NST // 8):
                    tp = ps_small.tile([P, 8 * P], F16, tag="small")
                    for j in range(8):
                        nc.tensor.transpose(
                            tp[:, j * P : (j + 1) * P],
                            E[:, (8 * sb + j) * P : (8 * sb + j + 1) * P],
                            identh[:],
                        )
                    nc.vector.tensor_copy(ET[:, 8 * sb : 8 * sb + 8, :], tp[:])
                return ET

            def emit_out_chunk(ET, st, qt, c):
                cs = slice(c * NCH, (c + 1) * NCH)
                po = ps_out.tile([P, NCH], F32, tag="po")
                for s_ in range(NST):
                    nc.tensor.matmul(
                        po[:],
                        ET[:, s_, :],
                        r32(enc_sb[s_][:, cs]),
                        start=(s_ == 0),
                        stop=(s_ == NST - 1),
                    )
                ob = outp.tile([P, NCH], F32, tag="ob")
                nc.vector.tensor_scalar_mul(ob[:], po[:], st[:, 2:3])
                nc.sync.dma_start(out=out[qt * P : (qt + 1) * P, cs], in_=ob[:])

            # ---------------- steady state
            _mark(nc, "scores(0)")
            cur_sc = sc0
            prev = None  # (ET, st, qt) of tile i-1
            for i in range(NQT):
                _mark(nc, f"softmax({i})")
                E, st = emit_softmax_stats(cur_sc)
                if prev is not None:
                    _mark(nc, f"out({i - 1})c23")
                    emit_out_chunk(*prev, 2)
                    emit_out_chunk(*prev, 3)
                _mark(nc, f"transp({i})")
                ET = emit_transp(E)
                nc.vector.tensor_reduce(
                    st[:, 1:2], st[:, 4:6], axis=AX.X, op=ALU.add
                )
                nc.vector.reciprocal(st[:, 2:3], st[:, 1:2])
                if i + 1 < NQT:
                    if (i + 1) % QTPG == 0:
                        _mark(nc, f"P1({(i + 1) // QTPG})")
                        projs[(i + 1) // QTPG] = emit_p1((i + 1) // QTPG)
                    _mark(nc, f"scores({i + 1})")
                    cur_sc = emit_scores(i + 1)
                _mark(nc, f"out({i})c01")
                emit_out_chunk(ET, st, i, 0)
                emit_out_chunk(ET, st, i, 1)
                prev = (ET, st, i)
            _mark(nc, "tail")
            emit_out_chunk(*prev, 2)
            ETp, stp, qtp = prev
            for half in range(2):
                hs = slice(
                    3 * NCH + half * (NCH // 2),
                    3 * NCH + (half + 1) * (NCH // 2),
                )
                po = ps_out.tile([P, NCH], F32, tag="po")
                for s_ in range(NST):
                    nc.tensor.matmul(
                        po[:, 0 : NCH // 2],
                        ETp[:, s_, :],
                        r32(enc_sb[s_][:, hs]),
                        start=(s_ == 0),
                        stop=(s_ == NST - 1),
                    )
                ob = outp.tile([P, NCH], F32, tag="ob")
                nc.vector.tensor_scalar_mul(
                    ob[:, 0 : NCH // 2], po[:, 0 : NCH // 2], stp[:, 2:3]
                )
                nc.sync.dma_start(
                    out=out[qtp * P : (qtp + 1) * P, hs], in_=ob[:, 0 : NCH // 2]
                )
            _mark(nc, "end")

    nc.compile()
    return nc


PHASES = []  # (instruction id, label) marks populated during build


def _mark(nc, label):
    nm = nc.get_next_instruction_name()  # burns one name; fine
    PHASES.append((int(nm.split("-")[1]), label))


_NC_CACHE = {}


def _get_program(loop_n: int = 1) -> bass.Bass:
    if loop_n not in _NC_CACHE:
        PHASES.clear()
        _NC_CACHE[loop_n] = build_program()
    return _NC_CACHE[loop_n]


def _prep_in_maps(input, encoder_output, W, b):
    input = np.ascontiguousarray(input, dtype=np.float32)
    encoder_output = np.ascontiguousarray(encoder_output, dtype=np.float32)
    W = np.ascontiguousarray(W, dtype=np.float32)
    b = np.ascontiguousarray(b, dtype=np.float32)

    # input per batch [TQ, H] -> inpT[g, hp, ht, q'] = input[g*QG+q', ht*P+hp]
    inpT = np.ascontiguousarray(
        input.reshape(B, NG, QG, NHT, P).transpose(0, 1, 4, 3, 2)
    ).astype(np.float16)
    # W[d, h] -> wt[dt, hp, ht, dj] = W[dt*P+dj, ht*P+hp], fp16
    wt = np.ascontiguousarray(
        W.reshape(NDT, P, NHT, P).transpose(0, 3, 2, 1)
    ).astype(np.float16)
    bvec = np.ascontiguousarray(b.reshape(NDT, P).T)  # [P, NDT] fp32

    enc16 = encoder_output.astype(np.float16)
    return [
        {"inpT": inpT[i], "wt": wt, "enc": enc16[i], "bvec": bvec}
        for i in range(B)
    ]


def _out_from_results(res):
    return np.stack([np.asarray(res.results[i]["out"]) for i in range(B)])


def run(input, encoder_output, W, b, trace=False, loop_n=1):
    """Returns (out [B, TQ, S] float32, BassKernelResults)."""
    nc = _get_program(loop_n)
    in_maps = _prep_in_maps(input, encoder_output, W, b)
    res = run_bass_kernel_spmd(nc, in_maps, list(range(B)), trace=trace)
    return _out_from_results(res), res


def kernel(input, encoder_output, W, b):
    out, _ = run(input, encoder_output, W, b, trace=False)
    return out

